# revision 1
# baseline (speedup 1.0000x reference)
"""DeepSeek MoE gate (noaux_tc routing) on 8 TRN2 NeuronCores.

Full inputs:
    hidden_states            [8192, 7168] f32
    weight                   [256, 7168]  f32
    e_score_correction_bias  [256]        f32
Full outputs (tuple, like the reference):
    routing_weights          [8192, 8] f32
    selected_experts         [8192, 8] int32

Sharding: token dim split 8 ways (1024 tokens/core); weight + bias replicated.

Matmul precision scheme (MOE_SCHEME):
  f32    — plain fp32 matmuls (HW runs 2 half-rate passes per matmul)
  f32r1  — single-pass float32r (fast, ~13-bit mantissa: top-k flips ~28 rows)
  f32r3  — xh@wh + xl@wh + xh@wl with f32r operands, where xh=f32r(x),
           xl=x-xh (rounding residuals). Error ~2^-26: fp32-class accuracy
           at 3 cycles/row instead of fp32's 4, with the residuals produced
           during the PSUM evacuation that the PE-transpose pipeline needs
           anyway.

Per-core pipeline:
  - build wT (+wlT for f32r3) once via PE transposes of the gate weight
  - stream x in half-tiles [128, 3584]; PE-transpose [128,128] blocks into
    PSUM pairs, evacuate: xh cast on ACT, xl residual on DVE
  - accumulate logits[t,e] in PSUM over 56 k-chunks (x-stationary matmuls)
  - epilogue per tile: sigmoid (ACT), +bias, group top-2 via nc.vector.max,
    top-4 groups by threshold, expert mask, top-8 via max/max_index,
    raw-score gather via value-match scalar_tensor_tensor, normalize, x2.5
"""

import os
import sys

import numpy as np

T_FULL = 8192
H = 7168
E = 256
N_CORES = 8
T_LOC = T_FULL // N_CORES          # 1024 tokens per core
P = 128                            # partition tile
N_TTILES = T_LOC // P              # 8 token tiles per core
N_K = H // P                       # 56 contraction chunks
HALF_K = N_K // 2                  # k-chunks per x half-tile
N_GROUP = 8
EG = E // N_GROUP                  # 32 experts per group
TOPK_GROUP = 4
TOP_K = 8
ROUTED_SCALING = 2.5

SCHEME = os.environ.get("MOE_SCHEME", "f32r3")
assert SCHEME in ("f32", "f32r1", "f32r3")


def _ensure_path():
    for p in ("/opt/trn_rl_repo", "/root/.axon_site/_ro/trn_rl_repo"):
        if os.path.isdir(p) and p not in sys.path:
            sys.path.append(p)


def _build_program():
    _ensure_path()
    import concourse.bass as bass  # noqa: F401
    import concourse.mybir as mybir
    from concourse import bacc
    from concourse.tile import TileContext

    f32 = mybir.dt.float32
    mm_dt = f32 if SCHEME == "f32" else mybir.dt.float32r
    residual = SCHEME == "f32r3"
    u32 = mybir.dt.uint32
    i32 = mybir.dt.int32
    Alu = mybir.AluOpType
    Act = mybir.ActivationFunctionType

    nc = bacc.Bacc("TRN2", debug=False, enable_asserts=False)

    hs = nc.dram_tensor("hidden_states", [T_LOC, H], f32, kind="ExternalInput")
    wt = nc.dram_tensor("weight", [E, H], f32, kind="ExternalInput")
    bias = nc.dram_tensor(
        "e_score_correction_bias", [E], f32, kind="ExternalInput"
    )
    out_w = nc.dram_tensor("routing_weights", [T_LOC, TOP_K], f32, kind="ExternalOutput")
    out_i = nc.dram_tensor("selected_experts", [T_LOC, TOP_K], i32, kind="ExternalOutput")

    with TileContext(nc) as tc:
        with (
            tc.tile_pool(name="const", bufs=1) as const_pool,
            tc.tile_pool(name="wT", bufs=1) as wT_pool,
            tc.tile_pool(name="x", bufs=2) as x_pool,
            tc.tile_pool(name="xt", bufs=6) as xt_pool,
            tc.tile_pool(name="ps_t", bufs=3, space="PSUM") as ps_t_pool,
            tc.tile_pool(name="ps_l", bufs=3, space="PSUM") as ps_l_pool,
            tc.tile_pool(name="epi", bufs=2) as epi_pool,
            tc.tile_pool(name="stage", bufs=1) as stage_pool,
        ):
            # ---- constants -------------------------------------------------
            eye_dram = nc.inline_tensor(np.eye(P, dtype=np.float32), name="eye128")
            identity = const_pool.tile([P, P], f32)
            nc.sync.dma_start(out=identity, in_=eye_dram.ap())

            ones_dram = nc.inline_tensor(
                np.ones((1, P), dtype=np.float32), name="ones128"
            )
            ones_row = const_pool.tile([1, P], f32)
            nc.sync.dma_start(out=ones_row, in_=ones_dram.ap())

            bias_row = const_pool.tile([1, E], f32)
            nc.sync.dma_start(
                out=bias_row, in_=bias.ap().rearrange("(o e) -> o e", o=1)
            )
            # broadcast bias across partitions: rank-1 matmul ones^T @ bias_row
            bias_bc = const_pool.tile([P, E], f32)
            ps_b = ps_l_pool.tile([P, E], f32, tag="ps_l")
            nc.tensor.matmul(ps_b, lhsT=ones_row, rhs=bias_row, start=True, stop=True)
            nc.vector.tensor_copy(bias_bc, ps_b)

            # ---- build wT [h, e] (+ wlT residual) --------------------------
            wT = wT_pool.tile([P, N_K, E], mm_dt)
            wlT = None
            if residual:
                wlT = wT_pool.tile([P, N_K, E], mm_dt, name="wlT")
            N_WCHUNK = 8
            KC = N_K // N_WCHUNK  # k-chunks per w load chunk
            with tc.tile_pool(name="wnat", bufs=2) as wnat_pool:
                for q in range(N_WCHUNK):
                    w_nat = wnat_pool.tile([P, 2, H // N_WCHUNK], f32, tag="wnat")
                    for eh in range(2):
                        nc.sync.dma_start(
                            out=w_nat[:, eh, :],
                            in_=wt.ap()[
                                eh * P : (eh + 1) * P,
                                q * (H // N_WCHUNK) : (q + 1) * (H // N_WCHUNK),
                            ],
                        )
                    for kk in range(KC):
                        k = q * KC + kk
                        pst = ps_t_pool.tile([P, 2 * P], f32, tag="ps_t")
                        for eh in range(2):
                            nc.tensor.transpose(
                                pst[:, eh * P : (eh + 1) * P],
                                w_nat[:, eh, kk * P : (kk + 1) * P],
                                identity,
                            )
                        nc.scalar.copy(wT[:, k, :], pst)
                        if residual:
                            # wl = w - f32r(w); exactly representable in f32r
                            # (tiny relative to w)
                            nc.vector.scalar_tensor_tensor(
                                out=wlT[:, k, :],
                                in0=wT[:, k, :],
                                scalar=-1.0,
                                in1=pst,
                                op0=Alu.mult,
                                op1=Alu.add,
                            )

            # ---- output staging -------------------------------------------
            stage_w = stage_pool.tile([P, N_TTILES, TOP_K], f32)
            stage_i = stage_pool.tile([P, N_TTILES, TOP_K], u32)

            # ---- main loop over token tiles --------------------------------
            for ti in range(N_TTILES):
                # x half-tiles so transposes can start after ~1.8MB lands
                x_half = []
                for h in range(2):
                    xha = x_pool.tile([P, H // 2], f32, tag="x", name=f"xh_{ti}_{h}")
                    x_half.append(xha)
                    nc.sync.dma_start(
                        out=xha,
                        in_=hs.ap()[
                            ti * P : (ti + 1) * P,
                            h * (H // 2) : (h + 1) * (H // 2),
                        ],
                    )

                ps_log = ps_l_pool.tile([P, E], f32, tag="ps_l")

                # software-pipelined: transpose pair pk+1 while pair pk matmuls
                n_pairs = N_K // 2
                pend = []  # (k0, xh_tile, xl_tile)
                for pk in range(n_pairs + 1):
                    if pk < n_pairs:
                        k0 = 2 * pk
                        pst = ps_t_pool.tile([P, 2 * P], f32, tag="ps_t")
                        for j in range(2):
                            k = k0 + j
                            src = x_half[k // HALF_K]
                            kk = k % HALF_K
                            nc.tensor.transpose(
                                pst[:, j * P : (j + 1) * P],
                                src[:, kk * P : (kk + 1) * P],
                                identity,
                            )
                        xh = xt_pool.tile([P, 2 * P], mm_dt, tag="xh")
                        nc.scalar.copy(xh, pst)  # ACT: rounds f32 -> mm dtype
                        if residual:
                            xl = xt_pool.tile([P, 2 * P], mm_dt, tag="xl")
                            nc.vector.scalar_tensor_tensor(
                                out=xl,
                                in0=xh,
                                scalar=-1.0,
                                in1=pst,
                                op0=Alu.mult,
                                op1=Alu.add,
                            )
                        else:
                            xl = None
                        pend.append((k0, xh, xl))
                    if pk >= 1:
                        k0, xh, xl = pend[pk - 1]
                        for j in range(2):
                            k = k0 + j
                            first = k == 0
                            last = k == N_K - 1
                            xh_j = xh[:, j * P : (j + 1) * P]
                            if residual:
                                xl_j = xl[:, j * P : (j + 1) * P]
                                nc.tensor.matmul(
                                    ps_log, lhsT=xh_j, rhs=wT[:, k, :],
                                    start=first, stop=False,
                                )
                                nc.tensor.matmul(
                                    ps_log, lhsT=xl_j, rhs=wT[:, k, :],
                                    start=False, stop=False,
                                )
                                nc.tensor.matmul(
                                    ps_log, lhsT=xh_j, rhs=wlT[:, k, :],
                                    start=False, stop=last,
                                )
                            else:
                                nc.tensor.matmul(
                                    ps_log, lhsT=xh_j, rhs=wT[:, k, :],
                                    start=first, stop=last,
                                )

                # ---- epilogue ---------------------------------------------
                scores = epi_pool.tile([P, E], f32, tag="scores")
                nc.scalar.activation(scores, ps_log, Act.Sigmoid)

                s_choice = epi_pool.tile([P, E], f32, tag="s_choice")
                nc.vector.tensor_add(s_choice, scores, bias_bc)

                # per-group top-8 (entries 0,1 used) -> group scores
                gmax = epi_pool.tile([P, N_GROUP, 8], f32, tag="gmax")
                for g in range(N_GROUP):
                    nc.vector.max(
                        out=gmax[:, g, :], in_=s_choice[:, g * EG : (g + 1) * EG]
                    )
                gscore = epi_pool.tile([P, N_GROUP], f32, tag="gscore")
                nc.vector.tensor_add(gscore, gmax[:, :, 0], gmax[:, :, 1])

                # top-4 groups: threshold at 4th largest group score
                g8 = epi_pool.tile([P, 8], f32, tag="g8")
                nc.vector.max(out=g8, in_=gscore)
                gmask = epi_pool.tile([P, N_GROUP], f32, tag="gmask")
                nc.vector.tensor_tensor(
                    out=gmask,
                    in0=gscore,
                    in1=g8[:, TOPK_GROUP - 1 : TOPK_GROUP].to_broadcast(
                        [P, N_GROUP]
                    ),
                    op=Alu.is_ge,
                )

                # expand to expert mask and apply
                emask = epi_pool.tile([P, E], f32, tag="emask")
                nc.vector.tensor_copy(
                    emask.rearrange("p (g x) -> p g x", g=N_GROUP),
                    gmask.rearrange("p (g x) -> p g x", x=1).to_broadcast(
                        [P, N_GROUP, EG]
                    ),
                )
                masked = epi_pool.tile([P, E], f32, tag="masked")
                nc.vector.tensor_mul(masked, s_choice, emask)

                # top-8 experts
                v8 = epi_pool.tile([P, 8], f32, tag="v8")
                nc.vector.max(out=v8, in_=masked)
                idx_u = epi_pool.tile([P, 8], u32, tag="idx_u")
                nc.vector.max_index(idx_u, v8, masked)

                # gather raw sigmoid scores at the top-8 positions by matching
                # each top value against the masked tensor (ties have ~0 prob)
                raw8 = epi_pool.tile([P, 8], f32, tag="raw8")
                for kk in range(TOP_K):
                    sc256 = epi_pool.tile([P, E], f32, tag="emask")
                    nc.vector.scalar_tensor_tensor(
                        out=sc256,
                        in0=masked,
                        scalar=v8[:, kk : kk + 1],
                        in1=scores,
                        op0=Alu.is_equal,
                        op1=Alu.mult,
                        accum_out=raw8[:, kk : kk + 1],
                    )

                # normalize * 2.5
                rsum = epi_pool.tile([P, 1], f32, tag="rsum")
                nc.vector.reduce_sum(rsum, raw8, axis=mybir.AxisListType.X)
                nc.vector.tensor_scalar(
                    rsum, rsum, 1.0 / ROUTED_SCALING, None, op0=Alu.mult
                )
                rcp = epi_pool.tile([P, 1], f32, tag="rcp")
                nc.vector.reciprocal(rcp, rsum)
                nc.scalar.mul(stage_w[:, ti, :], raw8, rcp)
                nc.vector.tensor_copy(stage_i[:, ti, :], idx_u)

            # ---- write outputs --------------------------------------------
            nc.sync.dma_start(
                out=out_w.ap().rearrange("(n p) k -> p n k", p=P), in_=stage_w
            )
            nc.sync.dma_start(
                out=out_i.ap().rearrange("(n p) k -> p n k", p=P).bitcast(u32),
                in_=stage_i,
            )

    nc.finalize()
    return nc


_NC_CACHE = {}


def _get_program():
    if SCHEME not in _NC_CACHE:
        _NC_CACHE[SCHEME] = _build_program()
    return _NC_CACHE[SCHEME]


def kernel(hidden_states, weight, e_score_correction_bias, _trace=False):
    _ensure_path()
    from concourse.bass_utils import run_bass_kernel_spmd

    hidden_states = np.ascontiguousarray(hidden_states, dtype=np.float32)
    weight = np.ascontiguousarray(weight, dtype=np.float32)
    e_score_correction_bias = np.ascontiguousarray(
        e_score_correction_bias, dtype=np.float32
    )

    nc = _get_program()
    in_maps = [
        {
            "hidden_states": hidden_states[i * T_LOC : (i + 1) * T_LOC],
            "weight": weight,
            "e_score_correction_bias": e_score_correction_bias,
        }
        for i in range(N_CORES)
    ]
    res = run_bass_kernel_spmd(
        nc, in_maps, core_ids=list(range(N_CORES)), trace=_trace
    )
    routing_weights = np.concatenate(
        [res.results[i]["routing_weights"] for i in range(N_CORES)], axis=0
    )
    selected_experts = np.concatenate(
        [res.results[i]["selected_experts"] for i in range(N_CORES)], axis=0
    )
    if _trace:
        return (routing_weights, selected_experts), res
    return routing_weights, selected_experts

